# revision 37
# baseline (speedup 1.0000x reference)
"""AgreementRouting (capsule dynamic routing) Trainium2 kernel.

Problem: u_predict [B=32,G=8,S=1152,O=10,D=16] f32, b_param [G,1,S,O] f32,
n_iterations=3.  Per (b,g): 3 routing iterations (softmax over O, weighted
sum over S, squash, agreement update), output = sum over G of v: [B,O,D].

Sharding: data-parallel over B across 8 cores (4 batches/core); each core
handles 32 independent (batch,group) subproblems ("bg").

Per-core layout (S-major): u_all [128, 32bg, 9t, 160(o,d)] bf16 SBUF-resident
(host pre-casts u to bf16 -- halves PCIe/DMA bytes and skips on-chip cast).
Logits b are split into two half tensors (16 bgs each) so iterations 0..1 can
software-pipeline: PE runs step (i) of one half while DVE runs step (ii) of
the other.

Steps per iteration r:
  softmax (batched per half): exp [ACT] -> group-sum [DVE] -> recip [DVE]
      -> c = e*rz [DVE scalar_tensor_tensor with step-0 broadcast AP] (bf16)
  (i) s[bg] = sum_s c*u: 9 accumulating matmuls [PE] per bg into psum packs
      [10, 3*160]; ACT copies pack->SBUF; diagonal extraction via 10 small
      SBUF->SBUF DMAs per pack (s[o,d] = pack[o, j*160+o*16+d]).
  squash (batched per half on [16,10,16]).
  (ii) r<2: broadcast v row to 128 partitions via ones-matmul [PE] ->
      bf16 copy [ACT] -> u*vb [DVE] -> grouped reduce_sum over d [DVE]
      -> b += bupd [DVE].
Output: sum over g via matmul with block-ones lhsT [32,4], DMA out.

Walrus in this container rejects instructions with >1 semaphore wait, so
_split_excess_waits moves extra waits onto same-engine NoOps post-Tile.
"""

import sys

for _p in ("/opt/trn_rl_repo", "/root/.axon_site/_ro/trn_rl_repo"):
    if _p not in sys.path:
        sys.path.insert(0, _p)

import ml_dtypes
import numpy as np

import concourse.bass as bass
import concourse.tile as tile
from concourse import mybir
from concourse.bass_utils import run_bass_kernel_spmd

# ---- problem constants (hardcoded per spec) ----
B, G, S, O, D = 32, 8, 1152, 10, 16
N_CORES = 8
BPC = B // N_CORES          # 4 batches per core
NBG = BPC * G               # 32 (batch, group) subproblems per core
T = S // 128                # 9 s-tiles
P = 128
OD = O * D                  # 160
N_ITER = 3
EPS = 1e-8
HBG = NBG // 4              # 8 bgs per pipelined chunk
NCHUNK = 4

F32 = mybir.dt.float32
BF16 = mybir.dt.bfloat16
AX = mybir.AxisListType
ALU = mybir.AluOpType
AF = mybir.ActivationFunctionType

PACK_BG = 3  # bgs per psum bank in step (i): [10, 3*160] <= 512 f32 cols


def _bcast_ap(ap, extra_dims):
    """Append broadcast (step-0) free dims to an AP view."""
    new = ap.ap.copy()
    for n in extra_dims:
        new = new + [[0, n]]
    return bass.AP(tensor=ap.tensor, offset=ap.offset, ap=new)


def _mid_bcast_ap(ap, pos, n):
    """Insert a step-0 dim at position pos (after partition dim)."""
    new = ap.ap.copy()
    new.insert(pos, [0, n])
    return bass.AP(tensor=ap.tensor, offset=ap.offset, ap=new)


MAX_WAITS = 1  # walrus codegen rejects instructions with more sem-waits


def _split_excess_waits(nc):
    """Move excess on_wait entries onto same-engine NoOps inserted before."""
    eng_map = {
        mybir.EngineType.DVE: nc.vector,
        mybir.EngineType.Activation: nc.scalar,
        mybir.EngineType.PE: nc.tensor,
        mybir.EngineType.Pool: nc.gpsimd,
        mybir.EngineType.SP: nc.sync,
    }
    for bb in nc.main_func.blocks:
        insts = list(bb.instructions)
        out = []
        changed = False
        for inst in insts:
            si = inst.sync_info
            waits = list(si.on_wait) if (si and si.on_wait) else []
            if len(waits) > MAX_WAITS:
                extra, keep = waits[:-MAX_WAITS], waits[-MAX_WAITS:]
                builder = eng_map[inst.engine]
                for i in range(0, len(extra), MAX_WAITS):
                    nop = builder.nop().ins
                    for blk in nc.main_func.blocks:
                        if blk.instructions and blk.instructions[-1] is nop:
                            blk.instructions.pop()
                            break
                    nop.engine = inst.engine
                    nop.sync_info = mybir.SyncInfo(
                        on_wait=extra[i:i + MAX_WAITS], on_update=[])
                    out.append(nop)
                inst.sync_info = mybir.SyncInfo(
                    on_wait=keep,
                    on_update=list(si.on_update) if si.on_update else [])
                changed = True
            out.append(inst)
        if changed:
            bb.instructions = out


def build_kernel():
    nc = bass.Bass()
    u_in = nc.dram_tensor("u", [BPC, G, S, O, D], BF16, kind="ExternalInput")
    bp_in = nc.dram_tensor("bp", [G, 1, S, O], F32, kind="ExternalInput")
    e4_in = nc.dram_tensor("e4", [NBG, BPC], F32, kind="ExternalInput")
    out_dram = nc.dram_tensor("out", [BPC, O, D], F32, kind="ExternalOutput")

    with tile.TileContext(nc) as tc:
        with (
            tc.tile_pool(name="persist", bufs=1) as persist,
            tc.tile_pool(name="sm", bufs=2) as sm,
            tc.tile_pool(name="small", bufs=2) as small,
            tc.tile_pool(name="sq", bufs=2) as sqp,
            tc.tile_pool(name="spk", bufs=3) as spkp,
            tc.tile_pool(name="vb", bufs=3) as vbp,
            tc.tile_pool(name="prod", bufs=3) as prodp,
            tc.tile_pool(name="bupd", bufs=3) as bupdp,
            tc.tile_pool(name="psum_s", bufs=4, space="PSUM") as psum_s,
            tc.tile_pool(name="psum_v", bufs=2, space="PSUM") as psum_v,
        ):
            u_tiles = [persist.tile([P, T, OD], BF16, name=f"u{bg}",
                                    tag=f"u{bg}") for bg in range(NBG)]
            b_half = [persist.tile([P, HBG, T, O], F32, name=f"b{h}",
                                   tag=f"b{h}") for h in range(NCHUNK)]
            ones_sb = persist.tile([P, P], F32)
            e4h = [persist.tile([HBG, BPC], F32, name=f"e4h{h}",
                                tag=f"e4h{h}") for h in range(NCHUNK)]

            nc.vector.memset(ones_sb[:], 1.0)
            for h in range(NCHUNK):
                nc.sync.dma_start(out=e4h[h][:],
                                  in_=e4_in[h * HBG:(h + 1) * HBG])

            # ---- b loads first (softmax can start early), then u loads in
            # bg order on alternating HWDGE queues so (i) overlaps the stream
            for h in range(NCHUNK):
                for bg in range(h * HBG, (h + 1) * HBG):
                    bi, g = bg // G, bg % G
                    q = nc.sync if bg % 2 == 0 else nc.scalar
                    bsrc = bp_in[g, 0].rearrange("(p t) o -> p t o", p=P)
                    q.dma_start(out=b_half[bg // HBG][:, bg % HBG], in_=bsrc)
                for bg in range(h * HBG, (h + 1) * HBG):
                    bi, g = bg // G, bg % G
                    src = u_in[bi, g].rearrange("(p t) o d -> p t (o d)", p=P)
                    (nc.sync if bg % 2 == 0 else nc.scalar).dma_start(
                        out=u_tiles[bg][:], in_=src)

            def softmax_half(r, h, n_loc):
                """exp/Z/recip/c for n_loc bgs of half h (batched ops)."""
                bh = b_half[h]
                e_h = sm.tile([P, n_loc * T, O], F32, name=f"e{h}", tag=f"e{h}")
                nc.scalar.activation(
                    out=e_h[:],
                    in_=bh[:, :n_loc].rearrange("p a t o -> p (a t) o"),
                    func=AF.Exp)
                z_h = small.tile([P, n_loc * T], F32, name=f"z{h}", tag=f"z{h}")
                nc.vector.reduce_sum(out=z_h[:], in_=e_h[:], axis=AX.X)
                nc.vector.reciprocal(z_h[:], z_h[:])
                c_h = sm.tile([P, n_loc, T, O], BF16, name=f"c{h}", tag=f"c{h}")
                nc.vector.scalar_tensor_tensor(
                    out=c_h[:].rearrange("p a t o -> p (a t) o"),
                    in0=e_h[:], scalar=1.0,
                    in1=_bcast_ap(z_h[:], [O]),
                    op0=ALU.mult, op1=ALU.mult)
                return c_h

            def step_i(r, h, c_h, n_loc, dq):
                """(i) matmuls + pack copy + diag extract for n_loc bgs of
                half h; returns s tile [n_loc, O, D]."""
                s_h = sqp.tile([HBG, O, D], F32, name=f"s{h}r{r}",
                               tag=f"s{h}")
                n_packs = (n_loc + PACK_BG - 1) // PACK_BG
                for pk in range(n_packs):
                    nbg = min(PACK_BG, n_loc - pk * PACK_BG)
                    spk = psum_s.tile([O, PACK_BG * OD], F32, tag="spack",
                                      name="spack")
                    for j in range(nbg):
                        bgl = pk * PACK_BG + j
                        bg = h * HBG + bgl
                        for t in range(T):
                            nc.tensor.matmul(
                                spk[:, j * OD:(j + 1) * OD],
                                lhsT=c_h[:, bgl, t],
                                rhs=u_tiles[bg][:, t],
                                start=(t == 0), stop=(t == T - 1),
                                skip_group_check=True)
                    s_packed = spkp.tile([O, PACK_BG * OD], F32,
                                         tag="spacked", name="spacked")
                    nc.scalar.copy(s_packed[:, :nbg * OD], spk[:, :nbg * OD])
                    fs = PACK_BG * OD
                    for o in range(O):
                        src = bass.AP(
                            tensor=s_packed.tensor,
                            offset=s_packed[:].offset + o * fs + o * D,
                            ap=[[fs, 1], [OD, nbg], [1, D]])
                        dq[(pk + o) % 2].dma_start(
                            out=s_h[pk * PACK_BG:pk * PACK_BG + nbg, o],
                            in_=src)
                return s_h

            def squash_half(r, h, s_h, n_loc):
                sq = sqp.tile([HBG, O, D], F32, name=f"sq{h}", tag=f"sq{h}")
                nc.vector.tensor_mul(sq[:n_loc], s_h[:n_loc], s_h[:n_loc])
                l2 = small.tile([HBG, O], F32, name=f"l2{h}", tag=f"l2{h}")
                nc.vector.reduce_sum(out=l2[:n_loc], in_=sq[:n_loc], axis=AX.X)
                rt = small.tile([HBG, O], F32, name=f"rt{h}", tag=f"rt{h}")
                nc.scalar.activation(out=rt[:n_loc], in_=l2[:n_loc],
                                     func=AF.Sqrt)
                den = small.tile([HBG, O], F32, name=f"dn{h}", tag=f"dn{h}")
                nc.vector.tensor_scalar_add(rt[:n_loc], rt[:n_loc], EPS)
                nc.vector.tensor_scalar_add(den[:n_loc], l2[:n_loc], 1.0)
                nc.vector.tensor_mul(den[:n_loc], den[:n_loc], rt[:n_loc])
                nc.vector.reciprocal(den[:n_loc], den[:n_loc])
                nc.vector.tensor_mul(den[:n_loc], l2[:n_loc], den[:n_loc])
                v_h = sqp.tile([HBG, O, D], F32, name=f"v{h}r{r}",
                               tag=f"v{h}")
                nc.vector.scalar_tensor_tensor(
                    out=v_h[:n_loc], in0=s_h[:n_loc], scalar=1.0,
                    in1=_bcast_ap(den[:n_loc], [D]),
                    op0=ALU.mult, op1=ALU.mult)
                return v_h

            def step_ii(r, h, v_h, n_loc):
                """b += sum_d u*v for the half's bgs."""
                n_packs = (n_loc + PACK_BG - 1) // PACK_BG
                v4 = sqp.tile([P, n_packs, OD], F32, name=f"v4{h}",
                              tag=f"v4{h}")
                for q in range(PACK_BG):
                    cnt = len(range(q, n_loc, PACK_BG))
                    if cnt == 0:
                        continue
                    src = bass.AP(
                        tensor=v_h.tensor,
                        offset=v_h[:].offset + q * OD,
                        ap=[[PACK_BG * OD, cnt], [1, OD]])
                    nc.sync.dma_start(out=v4[32 * q:32 * q + 1, :cnt],
                                      in_=src)
                for bgl in range(n_loc):
                    bg = h * HBG + bgl
                    q, pl = bgl % PACK_BG, bgl // PACK_BG
                    vb_ps = psum_v.tile([P, OD], F32, tag="vbps",
                                        name="vbps")
                    nc.tensor.matmul(
                        vb_ps[:], lhsT=ones_sb[32 * q:32 * q + 1, :],
                        rhs=v4[32 * q:32 * q + 1, pl],
                        start=True, stop=True)
                    vbc = vbp.tile([P, T, OD], BF16, tag="vbc", name="vbc")
                    nc.scalar.copy(vbc[:], _mid_bcast_ap(vb_ps[:], 1, T))
                    tp = prodp.tile([P, T, OD], BF16, tag="tp", name="tp")
                    nc.vector.tensor_mul(tp[:], u_tiles[bg][:], vbc[:])
                    bu = bupdp.tile([P, T * O], F32, tag="bu", name="bu")
                    nc.vector.reduce_sum(
                        out=bu[:],
                        in_=tp[:].rearrange("p t (o d) -> p (t o) d", o=O),
                        axis=AX.X)
                    bh = b_half[h]
                    nc.vector.tensor_add(
                        bh[:, bgl].rearrange("p t o -> p (t o)"),
                        bh[:, bgl].rearrange("p t o -> p (t o)"),
                        bu[:])

            dq = [nc.sync, nc.scalar]
            # ---- iterations: pipelined quarter-chunks ----
            v_fin = [None] * NCHUNK
            for r in range(N_ITER):
                for h in range(NCHUNK):
                    c_h = softmax_half(r, h, HBG)
                    s_h = step_i(r, h, c_h, HBG, dq)
                    v_h = squash_half(r, h, s_h, HBG)
                    if r < N_ITER - 1:
                        step_ii(r, h, v_h, HBG)
                    else:
                        v_fin[h] = v_h

            # ---- output: out[b] = sum_g v  (accumulating matmuls) ----
            out_ps = psum_v.tile([BPC, OD], F32, tag="outps", name="outps")
            for h in range(NCHUNK):
                nc.tensor.matmul(
                    out_ps[:], lhsT=e4h[h][:],
                    rhs=v_fin[h][:].rearrange("p a b -> p (a b)"),
                    start=(h == 0), stop=(h == NCHUNK - 1))
            out_sb = small.tile([BPC, O, D], F32, tag="outsb", name="outsb")
            nc.vector.tensor_copy(
                out_sb[:], out_ps[:].rearrange("p (o d) -> p o d", o=O))
            nc.sync.dma_start(out=out_dram[:], in_=out_sb[:])

    _split_excess_waits(nc)
    return nc


_NC_CACHE = {}


def _get_nc():
    if "nc" not in _NC_CACHE:
        _NC_CACHE["nc"] = build_kernel()
    return _NC_CACHE["nc"]


def kernel(u_predict, b_param, n_iterations, _trace=False):
    assert int(n_iterations) == N_ITER
    u = np.asarray(u_predict)
    bp = np.asarray(b_param, dtype=np.float32)
    u_bf = u.astype(ml_dtypes.bfloat16)
    nc = _get_nc()
    e4 = np.zeros((NBG, BPC), dtype=np.float32)
    for j in range(BPC):
        e4[j * G:(j + 1) * G, j] = 1.0
    in_maps = []
    for core in range(N_CORES):
        in_maps.append({
            "u": np.ascontiguousarray(u_bf[core * BPC:(core + 1) * BPC]),
            "bp": bp,
            "e4": e4,
        })
    res = run_bass_kernel_spmd(
        nc, in_maps, core_ids=list(range(N_CORES)), trace=_trace,
    )
    out = np.concatenate([res.results[c]["out"] for c in range(N_CORES)],
                         axis=0)
    if _trace:
        kernel.last_exec_time_ns = res.exec_time_ns
        kernel.last_results = res
    return out
